# revision 1
# baseline (speedup 1.0000x reference)
"""Trainium2 Bass kernel for the ContrastiveLearningModule loss.

Math (mirrors the reference):
  P = l2norm(relu(E @ W1.T + b1) @ W2.T + b2)  rowwise over [T,V,L,N,D]
  for each node type t, anchors idx[t][v,l,:]:
    pos  = sum_{(x,y) != (v,l)} exp(z . P[t,x,y,id]/TEMP)
    negi = sum_{s' != s}        exp(z . z_{s'}   /TEMP)
    negc = sum_{o,k}            exp(z . P[o,v,l,nid]/TEMP)
    loss = log(pos+negi+negc) - log(pos);  out = sum(loss)/1440

Key optimizations:
 - only the gathered rows (~10.8k of 96k) are ever projected.  The host
   shards by gathering each core's rows (3 of the 24 (t,v,l) anchor groups
   per core, padded to a uniform shape so all 8 cores run one SPMD program),
   pre-transposed to [D, cols] so the on-device matmuls need no transposes.
 - bf16 operands for all large matmuls/elementwise (fp32 PSUM accumulation);
   final rel err ~1e-5.
 - one ACT function table (relu/identity/exp/ln): 1/||z|| = exp(-0.5 ln n2)
   instead of sqrt+reciprocal, avoiding table reloads.
 - masking via host-built additive masks (exp underflows to exact 0).
Each core returns a partial loss sum; the host combines.
"""

import sys

import numpy as np

sys.path.insert(0, "/opt/trn_rl_repo")

import concourse.bacc as bacc
import concourse.bass as bass
import concourse.mybir as mybir
import concourse.tile as tile
from concourse.bass_utils import run_bass_kernel_spmd
from concourse.hw_specs import get_activation_tables as _real_gat

_ONE_TABLE = "natural_log_exp_and_others"  # holds relu/identity/exp/ln/copy


def _gat_one_table(arch):
    """Restrict the act-table-load pass to a single function set that covers
    every ACT func this kernel uses, so exactly one LoadActFuncSet is
    emitted (the greedy per-func chooser otherwise thrashes between the
    exp and ln tables every block).  Table ids stay valid because the
    list order/length is unchanged -- other sets are just emptied."""
    tabs = _real_gat(arch)
    if _ONE_TABLE in tabs:
        return {k: (v if k == _ONE_TABLE else set()) for k, v in tabs.items()}
    return tabs


bacc.get_activation_tables = _gat_one_table

F32 = mybir.dt.float32
BF16 = mybir.dt.bfloat16
AF = mybir.ActivationFunctionType
NP_BF16 = mybir.dt.np(BF16)

# Problem constants (hardcoded per harness contract).
T, V, L, N, D = 4, 2, 3, 4000, 256
TEMP = 0.5
S = 100          # padded anchors per group (prio groups are exactly 100)
KPAD = 50        # padded cross-negatives per other-type (prio exactly 50)
NK = 3 * KPAD    # 150 cross-negative columns per group
XY = V * L       # 6 (view, layer) slabs
GCOLS = XY * S   # 600 gathered positive columns per group
SLOT = 768       # column stride per group slot (600 + 150 + 18 pad)
NSLOT = 3        # groups per core
NCOL = SLOT * NSLOT  # 2304 packed columns per core
NCORES = 8
COUNT = 1440.0   # total anchor count in the reference loss
NEG_BIG = -15000.0  # additive mask; exp(2*(sims+NEG_BIG)) underflows to 0
MMW = 512  # max matmul moving free dim into one PSUM bank

_CACHE = {}


def _emit_body(nc, tc, consts, dram, pools, rep):
    """One full loss computation: per-slot projection + similarities,
    interleaved so slot s's similarity work overlaps slot s+1's
    projection."""
    w1, w2, bb, ones_col, ones_colf, ones_row, ones11, eps, mi, mcm, ms = consts
    xt_d, mk_d, out_d = dram
    ppool, xpool, wpool, spool, psmm, psnb, pssim = pools
    r = f"r{rep}"
    w = SLOT
    halves = [slice(h0, min(h0 + MMW, w)) for h0 in range(0, w, MMW)]

    if True:
        # masked per-anchor losses, one column per slot; summed at the end
        lm_all = spool.tile([S, NSLOT], F32, name=f"lm{r}", tag="lm")
        for s in range(NSLOT):
            o = s * SLOT
            # ---- Projection + normalization of slot s's 768 columns ----
            ph = [ppool.tile([128, SLOT], BF16, name=f"ph{j}_{s}{r}",
                             tag=f"ph{j}_{s}") for j in (0, 1)]
            xtile = xpool.tile([128, 2 * w], BF16, name="xtile", tag="xtile")
            nc.sync.dma_start(xtile[:], xt_d[:, 2 * o:2 * o + 2 * w])
            xs = [xtile[:, 0:w], xtile[:, w:2 * w]]
            if rep == 0 and s == 0:
                # masks are needed only by the sims stage; load them after
                # the first projection inputs so compute starts sooner
                nc.sync.dma_start(mi.tensor.ap()[0:S, :], mk_d[:])
            hs = []
            for jt in (0, 1):
                psh = psmm.tile([128, w], F32, name="ps", tag="ps")
                for sl in halves:
                    nc.tensor.matmul(psh[:, sl], w1[0][:, jt * 128:(jt + 1) * 128],
                                     xs[0][:, sl], start=True, stop=False)
                    nc.tensor.matmul(psh[:, sl], w1[1][:, jt * 128:(jt + 1) * 128],
                                     xs[1][:, sl], start=False, stop=True)
                h = wpool.tile([128, w], BF16, name=f"h{jt}", tag=f"h{jt}")
                nc.scalar.activation(h[:], psh[:], AF.Relu, bias=bb[:, jt:jt + 1])
                hs.append(h)
            zs = []
            sqs = []
            for jt in (0, 1):
                psz = psmm.tile([128, w], F32, name="ps", tag="ps")
                for sl in halves:
                    nc.tensor.matmul(psz[:, sl], w2[0][:, jt * 128:(jt + 1) * 128],
                                     hs[0][:, sl], start=True, stop=False)
                    nc.tensor.matmul(psz[:, sl], w2[1][:, jt * 128:(jt + 1) * 128],
                                     hs[1][:, sl], start=False, stop=True)
                # z = psum + b2 on DVE (keeps ACT free for ln/exp)
                z = wpool.tile([128, w], BF16, name=f"z{jt}", tag=f"z{jt}")
                nc.vector.tensor_scalar_add(z[:], psz[:], bb[:, 2 + jt:3 + jt])
                zs.append(z)
                sq = wpool.tile([128, w], BF16, name=f"sq{jt}", tag=f"sq{jt}")
                nc.vector.tensor_mul(sq[:], z[:], z[:])
                sqs.append(sq)
            # ||z||^2 per column via ones-matmul partition reduction
            psn = psnb.tile([1, w], F32, name="psn", tag="pnb")
            for sl in halves:
                nc.tensor.matmul(psn[:, sl], ones_col[:], sqs[0][:, sl],
                                 start=True, stop=False)
                nc.tensor.matmul(psn[:, sl], ones_col[:], sqs[1][:, sl],
                                 start=False, stop=True)
            # 1/||z|| = exp(-0.5 ln(n2)) -- keeps exp/ln/relu in one table
            lnn = wpool.tile([1, w], F32, name="lnn", tag="lnn")
            nc.scalar.activation(lnn[:], psn[:], AF.Ln, bias=eps[:])
            rn = wpool.tile([1, w], BF16, name="rn", tag="rn")
            nc.scalar.activation(rn[:], lnn[:], AF.Exp, scale=-0.5)
            # broadcast 1/||z|| across partitions via rank-1 matmul
            psb = psnb.tile([128, w], F32, name="psb", tag="pnb")
            for sl in halves:
                nc.tensor.matmul(psb[:, sl], ones_row[:], rn[:, sl],
                                 start=True, stop=True)
            for jt in (0, 1):
                nc.vector.tensor_mul(ph[jt][:], zs[jt][:], psb[:])

            # ---- Similarities + loss for slot s ----
            # within-type sims: Z^T Z  [S, S]
            pin = pssim.tile([S, S], F32, name="pin", tag="sim")
            nc.tensor.matmul(pin[:], ph[0][:, 0:S], ph[0][:, 0:S],
                             start=True, stop=False)
            nc.tensor.matmul(pin[:], ph[1][:, 0:S], ph[1][:, 0:S],
                             start=False, stop=True)
            mski = spool.tile([S, S], F32, name="mski", tag="mski")
            nc.vector.tensor_add(mski[:], pin[:], mi[:, s * S:(s + 1) * S])
            ein = spool.tile([S, S], F32, name="ein", tag="ein")
            negin = spool.tile([S, 1], F32, name="negin", tag="negin")
            nc.scalar.activation(ein[:], mski[:], AF.Exp, scale=2.0,
                                 accum_out=negin[:])
            # cross-type sims: Z^T Nmat  [S, NK]
            pc = pssim.tile([S, NK], F32, name="pc", tag="sim")
            nc.tensor.matmul(pc[:], ph[0][:, 0:S], ph[0][:, GCOLS:GCOLS + NK],
                             start=True, stop=False)
            nc.tensor.matmul(pc[:], ph[1][:, 0:S], ph[1][:, GCOLS:GCOLS + NK],
                             start=False, stop=True)
            mskc = spool.tile([S, NK], F32, name="mskc", tag="mskc")
            nc.vector.tensor_add(mskc[:], pc[:], mcm[:, s * NK:(s + 1) * NK])
            ec = spool.tile([S, NK], F32, name="ec", tag="ec")
            negc = spool.tile([S, 1], F32, name="negc", tag="negc")
            nc.scalar.activation(ec[:], mskc[:], AF.Exp, scale=2.0,
                                 accum_out=negc[:])
            # positives: per-anchor dot with same node at other (x,y).
            # PR[:, xy*S+s] = ph[:, (xy+1)*S+s] * Z[:, s]; column-sum via
            # ones-matmul -> [1, 5*S] sims, exp, reduce over xy.
            ppr = pssim.tile([1, (XY - 1) * S], F32, name="ppr", tag="sim")
            for jt in (0, 1):
                pr = spool.tile([128, (XY - 1) * S], BF16, name="pr", tag="pr")
                zb = ph[jt][:, 0:S].unsqueeze(1).to_broadcast([128, XY - 1, S])
                nc.vector.tensor_mul(
                    pr[:].rearrange("p (a b) -> p a b", a=XY - 1),
                    ph[jt][:, S:XY * S].rearrange("p (a b) -> p a b", a=XY - 1),
                    zb)
                nc.tensor.matmul(ppr[:], ones_col[:], pr[:],
                                 start=(jt == 0), stop=(jt == 1))
            epr = spool.tile([1, (XY - 1) * S], F32, name="epr", tag="epr")
            nc.scalar.activation(epr[:], ppr[:], AF.Exp, scale=2.0)
            # pos[s] = sum_xy epr[xy*S+s]: 5 accumulating rank-1 transposes
            psp = pssim.tile([S, 1], F32, name="psp", tag="sim")
            for xy in range(XY - 1):
                nc.tensor.matmul(psp[:], epr[:, xy * S:(xy + 1) * S], ones11[:],
                                 start=(xy == 0), stop=(xy == XY - 2))
            # loss_s = ln(pos+neg) - ln(pos), then mask+sum via matmul
            neg = spool.tile([S, 1], F32, name="neg", tag="neg")
            nc.vector.tensor_add(neg[:], negin[:], negc[:])
            den = spool.tile([S, 1], F32, name="den", tag="den")
            nc.vector.tensor_add(den[:], neg[:], psp[:])
            lnden = spool.tile([S, 1], F32, name="lnden", tag="lnden")
            nc.scalar.activation(lnden[:], den[:], AF.Ln)
            lnpos = spool.tile([S, 1], F32, name="lnpos", tag="lnpos")
            nc.scalar.activation(lnpos[:], psp[:], AF.Ln)
            lossv = spool.tile([S, 1], F32, name="lossv", tag="lossv")
            nc.vector.tensor_sub(lossv[:], lnden[:], lnpos[:])
            nc.vector.tensor_mul(lm_all[:, s:s + 1], lossv[:], ms[:, s:s + 1])
        tot = pssim.tile([1, NSLOT], F32, name="tot", tag="sim")
        nc.tensor.matmul(tot[:], ones_colf[:S, :], lm_all[:], start=True, stop=True)
        osb = spool.tile([1, 1], F32, name="osb", tag="osb")
        nc.vector.reduce_sum(osb[:], tot[:], axis=mybir.AxisListType.X)
        nc.sync.dma_start(out_d[:], osb[:])


def _build_nc(reps=1, loop_iters=None):
    nc = bacc.Bacc("TRN2", target_bir_lowering=False, debug=False)

    xt_d = nc.dram_tensor("xt", [128, 2 * NCOL], BF16, kind="ExternalInput")
    wp_d = nc.dram_tensor("wp", [128, 4 * D], BF16, kind="ExternalInput")
    bb_d = nc.dram_tensor("bb", [128, 4], F32, kind="ExternalInput")
    mk_d = nc.dram_tensor("mk", [S, NSLOT * (S + NK + 1)], F32, kind="ExternalInput")
    out_d = nc.dram_tensor("out", [1, 1], F32, kind="ExternalOutput")

    with tile.TileContext(nc) as tc:
        with tc.tile_pool(name="const", bufs=1) as cpool:
            wtile = cpool.tile([128, 4 * D], BF16, name="wtile", tag="wtile")
            nc.sync.dma_start(wtile[:], wp_d[:])
            w1 = [wtile[:, 0:D], wtile[:, D:2 * D]]
            w2 = [wtile[:, 2 * D:3 * D], wtile[:, 3 * D:4 * D]]
            bb = cpool.tile([128, 4], F32, name="bb", tag="bb")
            nc.sync.dma_start(bb[:], bb_d[:])
            ones_col = cpool.tile([128, 1], BF16, name="ones_col", tag="ones_col")
            nc.vector.memset(ones_col[:], 1.0)
            ones_colf = cpool.tile([128, 1], F32, name="ones_colf", tag="ones_colf")
            nc.vector.memset(ones_colf[:], 1.0)
            ones_row = cpool.tile([1, 128], BF16, name="ones_row", tag="ones_row")
            nc.vector.memset(ones_row[:], 1.0)
            ones11 = cpool.tile([1, 1], F32, name="ones11", tag="ones11")
            nc.vector.memset(ones11[:], 1.0)
            eps = cpool.tile([1, 1], F32, name="eps", tag="eps")
            nc.vector.memset(eps[:], 1e-24)
            mk = cpool.tile([S, NSLOT * (S + NK + 1)], F32, name="mk", tag="mk")
            mi = mk[:, 0:NSLOT * S]
            mcm = mk[:, NSLOT * S:NSLOT * (S + NK)]
            ms = mk[:, NSLOT * (S + NK):]

            consts = (w1, w2, bb, ones_col, ones_colf, ones_row, ones11, eps,
                      mi, mcm, ms)
            dram = (xt_d, mk_d, out_d)
            with (
                tc.tile_pool(name="phat", bufs=2) as ppool,
                tc.tile_pool(name="xin", bufs=3) as xpool,
                tc.tile_pool(name="work", bufs=3) as wpool,
                tc.tile_pool(name="sbs", bufs=3) as spool,
                tc.tile_pool(name="psmm", bufs=2, space=bass.MemorySpace.PSUM) as psmm,
                tc.tile_pool(name="psnb", bufs=1, space=bass.MemorySpace.PSUM) as psnb,
                tc.tile_pool(name="pss", bufs=2, space=bass.MemorySpace.PSUM) as pssim,
            ):
                pools = (ppool, xpool, wpool, spool, psmm, psnb, pssim)
                if loop_iters is not None:
                    # device-side loop for wall-clock benchmarking
                    with tc.For_i(0, loop_iters, 1,
                                  hint_engines=(mybir.EngineType.PE,
                                                mybir.EngineType.DVE,
                                                mybir.EngineType.Activation)):
                        _emit_body(nc, tc, consts, dram, pools, 0)
                else:
                    for rep in range(reps):
                        _emit_body(nc, tc, consts, dram, pools, rep)

    nc.compile()
    return nc


def _get_nc(reps=1, loop_iters=None):
    key = ("nc", reps, loop_iters)
    if key not in _CACHE:
        _CACHE[key] = _build_nc(reps, loop_iters)
    return _CACHE[key]


def _groups():
    gs = [(t, v, l) for t in range(T) for v in range(V) for l in range(L)]
    return [[gs[c], gs[c + NCORES], gs[c + 2 * NCORES]] for c in range(NCORES)]


def make_in_maps(node_embeddings, W1, b1, W2, b2, idx_prio, idx_rest,
                 neg_idx_prio, neg_idx_rest):
    E = np.asarray(node_embeddings, dtype=np.float32)
    W1 = np.asarray(W1, dtype=np.float32)
    b1 = np.asarray(b1, dtype=np.float32)
    W2 = np.asarray(W2, dtype=np.float32)
    b2 = np.asarray(b2, dtype=np.float32)
    idxp = np.asarray(idx_prio)
    idxr = np.asarray(idx_rest)
    nidxp = np.asarray(neg_idx_prio)
    nidxr = np.asarray(neg_idx_rest)

    w1t = W1.T
    w2t = W2.T
    # packed weights: [128, 4D] = [w1t rows 0:128 | w1t rows 128:256 |
    #                              w2t rows 0:128 | w2t rows 128:256]
    wp = np.concatenate([w1t[:128], w1t[128:], w2t[:128], w2t[128:]],
                        axis=1).astype(NP_BF16)
    wp = np.ascontiguousarray(wp)
    bbm = np.stack([b1[:128], b1[128:], b2[:128], b2[128:]], axis=1)
    bbm = np.ascontiguousarray(bbm, dtype=np.float32)

    in_maps = []
    for gs in _groups():
        X = np.empty((NCOL, D), np.float32)
        MI = np.full((S, NSLOT * S), NEG_BIG, np.float32)
        MC = np.full((S, NSLOT * NK), NEG_BIG, np.float32)
        MS = np.zeros((S, NSLOT), np.float32)
        for si, (t, v, l) in enumerate(gs):
            if t < 2:
                idx, nid, Sr, Kr = idxp[t], nidxp[t], 100, 50
            else:
                idx, nid, Sr, Kr = idxr[t - 2], nidxr[t - 2], 20, 10
            ids = np.asarray(idx[v, l])
            ids_p = np.concatenate([ids, np.full(S - Sr, ids[0], ids.dtype)])
            o = si * SLOT
            xy_list = [(v, l)] + [(x, y) for x in range(V) for y in range(L)
                                  if (x, y) != (v, l)]
            for j, (x, y) in enumerate(xy_list):
                X[o + j * S:o + (j + 1) * S] = E[t, x, y, ids_p]
            others = [u for u in range(T) if u != t]
            for oi, u in enumerate(others):
                nk = np.asarray(nid[v, l, oi])
                nk_p = np.concatenate([nk, np.full(KPAD - Kr, nk[0], nk.dtype)])
                X[o + GCOLS + oi * KPAD:o + GCOLS + (oi + 1) * KPAD] = E[u, v, l, nk_p]
            X[o + GCOLS + NK:o + SLOT] = X[o]  # pad columns: dup of row 0
            # within-type mask: valid co-anchor and not the same sample
            MI[:, si * S:si * S + Sr] = 0.0
            MI[np.arange(S), si * S + np.arange(S)] = NEG_BIG
            # cross-type mask: valid negative columns
            for oi in range(3):
                MC[:, si * NK + oi * KPAD:si * NK + oi * KPAD + Kr] = 0.0
            MS[:Sr, si] = 1.0
        # xt packed per slot: [128, 2*NCOL], cols s*1536+j*768+c =
        # X.T[j*128+p, s*768+c]
        XT = X.T.astype(NP_BF16)
        XP = np.empty((128, 2 * NCOL), NP_BF16)
        for s in range(NSLOT):
            for j in (0, 1):
                XP[:, s * 2 * SLOT + j * SLOT:(s * 2 + j + 1) * SLOT] = \
                    XT[j * 128:(j + 1) * 128, s * SLOT:(s + 1) * SLOT]
        MK = np.concatenate([MI, MC, MS], axis=1)
        in_maps.append({
            "xt": np.ascontiguousarray(XP),
            "wp": wp, "bb": bbm,
            "mk": np.ascontiguousarray(MK),
        })
    return in_maps


def _make_runner(nc):
    """Lower nc to a cached jitted SPMD executable (mirrors
    bass2jax.run_bass_via_pjrt, but reusable across calls so repeat
    executions skip tracing/compilation)."""
    import jax
    from jax.experimental.shard_map import shard_map
    from jax.sharding import Mesh, PartitionSpec

    from concourse import bass2jax
    from concourse import mybir as mb

    bass2jax.install_neuronx_cc_hook()
    partition_name = (nc.partition_id_tensor.name
                      if nc.partition_id_tensor else None)
    in_names, out_names, out_avals = [], [], []
    for alloc in nc.m.functions[0].allocations:
        if not isinstance(alloc, mb.MemoryLocationSet):
            continue
        name = alloc.memorylocations[0].name
        if alloc.kind == "ExternalInput":
            if name != partition_name:
                in_names.append(name)
        elif alloc.kind == "ExternalOutput":
            out_names.append(name)
            out_avals.append(jax.core.ShapedArray(
                tuple(alloc.tensor_shape), mb.dt.np(alloc.dtype)))
    n_params = len(in_names)
    n_outs = len(out_avals)
    all_in_names = list(in_names) + list(out_names)
    if partition_name is not None:
        all_in_names.append(partition_name)

    def _body(*args):
        operands = list(args)
        if partition_name is not None:
            operands.append(bass2jax.partition_id_tensor())
        return tuple(bass2jax._bass_exec_p.bind(
            *operands,
            out_avals=tuple(out_avals),
            in_names=tuple(all_in_names),
            out_names=tuple(out_names),
            lowering_input_output_aliases=(),
            sim_require_finite=True,
            sim_require_nnan=True,
            nc=nc,
        ))

    devices = jax.devices()[:NCORES]
    mesh = Mesh(np.asarray(devices), ("core",))
    donate = tuple(range(n_params, n_params + n_outs))
    sharded = jax.jit(
        shard_map(_body, mesh=mesh,
                  in_specs=(PartitionSpec("core"),) * (n_params + n_outs),
                  out_specs=(PartitionSpec("core"),) * n_outs,
                  check_rep=False),
        donate_argnums=donate, keep_unused=True)

    def run(in_maps, device_inputs=None):
        if device_inputs is None:
            device_inputs = [
                np.concatenate([np.asarray(m[name]) for m in in_maps], axis=0)
                for name in in_names]
        zeros = [np.zeros((NCORES * a.shape[0], *a.shape[1:]), a.dtype)
                 for a in out_avals]
        out_arrs = sharded(*device_inputs, *zeros)
        return [
            {name: np.asarray(out_arrs[i]).reshape(NCORES, *out_avals[i].shape)[c]
             for i, name in enumerate(out_names)}
            for c in range(NCORES)
        ]

    run.in_names = in_names
    run.mesh = mesh
    return run


def _get_runner(reps=1, loop_iters=None):
    key = ("runner", reps, loop_iters)
    if key not in _CACHE:
        _CACHE[key] = _make_runner(_get_nc(reps, loop_iters))
    return _CACHE[key]


class _Res:
    def __init__(self, results):
        self.results = results


def run_on_hw(in_maps, reps=1, device_inputs=None, loop_iters=None):
    runner = _get_runner(reps, loop_iters)
    return _Res(runner(in_maps, device_inputs=device_inputs))


def kernel(node_embeddings, W1, b1, W2, b2, idx_prio, idx_rest,
           neg_idx_prio, neg_idx_rest, num_views=2, num_layers=3):
    in_maps = make_in_maps(node_embeddings, W1, b1, W2, b2, idx_prio, idx_rest,
                           neg_idx_prio, neg_idx_rest)
    res = run_on_hw(in_maps)
    _CACHE["last_results"] = res
    total = sum(float(res.results[c]["out"][0, 0]) for c in range(NCORES))
    return np.float32(total / COUNT)



# revision 52
# speedup vs baseline: 1.8126x; 1.8126x over previous
"""Trainium2 Bass kernel for the ContrastiveLearningModule loss.

Math (mirrors the reference):
  P = l2norm(relu(E @ W1.T + b1) @ W2.T + b2)  rowwise over [T,V,L,N,D]
  for each node type t, anchors idx[t][v,l,:]:
    pos  = sum_{(x,y) != (v,l)} exp(z . P[t,x,y,id]/TEMP)
    negi = sum_{s' != s}        exp(z . z_{s'}   /TEMP)
    negc = sum_{o,k}            exp(z . P[o,v,l,nid]/TEMP)
    loss = log(pos+negi+negc) - log(pos);  out = sum(loss)/1440

Sharding (no padding, all 8 cores run the identical program shape):
  24 (t,v,l) anchor groups -> per core: 1 full prio group (A: 100 anchors),
  1 half prio group (B: 50), 1 full rest group (C: 20), 1 half rest (D: 10).
  Per-core packed projection stream = exactly 1500 real columns:
    A: [anch 100 | negs 150 | pos 5x100]           cols    0:750
    B: [anch 50 | other-half 50 | negs 150 | pos 5x50]  750:1250
    C: [anch 20 | negs 30 | pos 5x20]                  1250:1400
    D: [anch 10 | other 10 | negs 30 | pos 5x10]       1400:1500
  No masks anywhere: the within-type "exclude self" term equals exp(2)
  exactly after normalization and is subtracted via the Ln bias; in-type
  partners + cross negs are one contiguous window per slot, so one
  exp(+accum) per slot yields the whole negative sum.

Device pipeline (per core):
  3 column-chunks of 512/512/476: L1 matmul -> relu (ACT/DVE split) ->
  L2 -> bias drains (Pool/DVE) -> sq (DVE) -> sqsum (Pool) ->
  psn = allones@sqsum (PE broadcast col-norms) -> ln/exp (ACT) -> ph (DVE).
  Sims: per slot one [Sa, win] matmul pair + exp(scale=2, accum) -> negsum.
  Pos: ph_pos * anchor-broadcast (DVE) -> ones-colsum (PE) -> exp ->
  5 rank-1 transposes accumulate pos[Sa,1]. Tail: den=negsum+pos,
  ln(den - e^2) - ln(pos), masked-free row sums via per-slot ones matmuls.
"""

import sys

import numpy as np

sys.path.insert(0, "/opt/trn_rl_repo")

import concourse.bacc as bacc
import concourse.bass as bass
import concourse.mybir as mybir
import concourse.tile as tile
from concourse.bass_utils import run_bass_kernel_spmd  # noqa: F401 (contract)
from concourse.hw_specs import get_activation_tables as _real_gat

_ONE_TABLE = "natural_log_exp_and_others"  # relu/identity/exp/ln/square


def _gat_one_table(arch):
    """Pin the act-table chooser to a single set covering every ACT func we
    use so exactly one LoadActFuncSet is emitted."""
    tabs = _real_gat(arch)
    if _ONE_TABLE in tabs:
        return {k: (v if k == _ONE_TABLE else set()) for k, v in tabs.items()}
    return tabs


bacc.get_activation_tables = _gat_one_table

F32 = mybir.dt.float32
BF16 = mybir.dt.bfloat16
AF = mybir.ActivationFunctionType
ALU = mybir.AluOpType
NP_BF16 = mybir.dt.np(BF16)

# Problem constants (hardcoded per harness contract).
T, V, L, N, D = 4, 2, 3, 4000, 256
TEMP = 0.5
XY = V * L                      # 6 (view, layer) slabs
NCORES = 8
COUNT = 1440.0                  # total anchors in the reference loss
E2 = float(np.exp(2.0))         # self-similarity term exp(sim(z,z)/TEMP)
NC = 1500                       # packed columns per core
# column layout: [anchors 240 | pos 900 | negs 360]; the negs go last so
# the post-projection tail after the final chunk is only the (short)
# negative-sims chain -- the pos chain completes during chunk 3's compute.
CHUNKS = [(0, 512), (512, 1024), (1024, 1140), (1140, 1500)]

# slots: (name, anch, Sa, ewin, ewl, pos, negs, nwl, prb, eb)
#   anch: anchor col base; ewin/ewl: early sims window (anchors+partners)
#   pos: 5*Sa pos block base; negs/nwl: negatives window
#   prb: offset of slot's pos-product block inside pr (per jt)
#   eb: offset inside the slot's epr row tile (A separate, BCD packed)
SLOT_A = ("A", 0, 100, 0, 100, 240, 1140, 150, 0, 0)
SLOT_B = ("B", 100, 50, 100, 100, 740, 1290, 150, 500, 0)
SLOT_C = ("C", 200, 20, 200, 20, 990, 1440, 30, 750, 250)
SLOT_D = ("D", 220, 10, 220, 20, 1090, 1470, 30, 850, 350)
SLOTS = [SLOT_A, SLOT_B, SLOT_C, SLOT_D]
PRW = 900                       # pos columns per jt (500+250+100+50)

_CACHE = {}


def _emit_body(nc, tc, consts, dram, pools, rep, emit_w2=False):
    wt, bbt, ones128, onesf, ones11, epsb, ne2b = consts
    xt_d, wp_d, out_d = dram
    (xpool, hpool, zpool, sqpool, sspool, lnpool, rnpool, phpool, prpool,
     escpool, smallpool, psL, psM, psS) = pools
    r = f"r{rep}"

    # w blocks: w1[(cin,jout)] = wt[:, (2*jout+cin)*128 : +128], w2 at +512
    def w1blk(cin, jout):
        o = (2 * jout + cin) * 128
        return wt[:, o:o + 128]

    def w2blk(cin, jout):
        o = 512 + (2 * jout + cin) * 128
        return wt[:, o:o + 128]

    z = zpool.tile([128, 2 * NC], BF16, name=f"z{r}", tag="z")
    ph = phpool.tile([128, 2 * NC], BF16, name=f"ph{r}", tag="ph")
    rn = rnpool.tile([128, 1536], BF16, name=f"rn{r}", tag="rn")
    pr = prpool.tile([128, 2 * PRW], BF16, name=f"pr{r}", tag="pr")
    nse = smallpool.tile([100, 4], F32, name=f"nse{r}", tag="nse")
    nsl = smallpool.tile([100, 4], F32, name=f"nsl{r}", tag="nsl")
    nst = smallpool.tile([100, 4], F32, name=f"nst{r}", tag="nst")
    den = smallpool.tile([100, 4], F32, name=f"den{r}", tag="den")
    lnden = smallpool.tile([100, 4], F32, name=f"lnden{r}", tag="lnden")
    lnpos = smallpool.tile([100, 4], F32, name=f"lnpos{r}", tag="lnpos")
    lossv = smallpool.tile([100, 4], F32, name=f"lossv{r}", tag="lossv")
    eprA = smallpool.tile([1, 500], F32, name=f"eprA{r}", tag="eprA")
    eprR = smallpool.tile([1, 400], F32, name=f"eprR{r}", tag="eprR")

    simps = {}
    # col offsets of each slot's region inside the merged sims tiles
    EOFF = [0, 100, 200, 220]    # early widths 100,100,20,20 -> 240
    LOFF = [0, 150, 300, 330]    # late  widths 150,150,30,30 -> 360

    def emit_sims_mm(si, late):
        """Sims matmul pair for slot si into the merged phase tile."""
        nm, anch, Sa, ewin, ewl, _, negs, nwl, _, _ = SLOTS[si]
        base, wl = (negs, nwl) if late else (ewin, ewl)
        off = (LOFF if late else EOFF)[si]
        key = ("L" if late else "E")
        if key not in simps:
            simps[key] = psS.tile([100, 384], F32, name=f"sim{key}{r}",
                                  tag="sim")
        ps = simps[key]
        nc.tensor.matmul(ps[0:Sa, off:off + wl], ph[:, anch:anch + Sa],
                         ph[:, base:base + wl], start=True, stop=False)
        nc.tensor.matmul(ps[0:Sa, off:off + wl],
                         ph[:, NC + anch:NC + anch + Sa],
                         ph[:, NC + base:NC + base + wl], start=False,
                         stop=True)

    def emit_sims_exp(late):
        """One exp over the merged tile + per-slot DVE row sums."""
        key = ("L" if late else "E")
        tw = 360 if late else 240
        ps = simps[key]
        esc = escpool.tile([100, 384], BF16, name=f"esc{key}{r}", tag="esc")
        nc.scalar.activation(esc[:, 0:tw], ps[:, 0:tw], AF.Exp, scale=2.0)
        acc = nsl if late else nse
        offs = LOFF if late else EOFF
        for si, sl in enumerate(SLOTS):
            Sa, wl = sl[2], (sl[7] if late else sl[4])
            nc.vector.reduce_sum(acc[0:Sa, si:si + 1],
                                 esc[0:Sa, offs[si]:offs[si] + wl],
                                 axis=mybir.AxisListType.X)

    def emit_pr(si):
        """Pos products: ph_pos * broadcast(anchors) for both jt.
        Slot A runs on Pool (its chain has the most slack); rest on DVE."""
        nm, anch, Sa, _, _, pos, _, _, prb, _ = SLOTS[si]
        eng = nc.gpsimd if si == 0 else nc.vector
        for jt in (0, 1):
            zb = ph[:, jt * NC + anch:jt * NC + anch + Sa] \
                .unsqueeze(1).to_broadcast([128, 5, Sa])
            eng.tensor_mul(
                pr[:, jt * PRW + prb:jt * PRW + prb + 5 * Sa]
                .rearrange("p (a b) -> p a b", a=5),
                ph[:, jt * NC + pos:jt * NC + pos + 5 * Sa]
                .rearrange("p (a b) -> p a b", a=5),
                zb)

    # ---- software-pipelined projection over 3 chunks ----
    # stage skew: front(t) | back(t-1) | norm(t-2) so every engine's
    # in-order queue sees ops in (approximate) execution-time order.
    NCH = len(CHUNKS)

    def emit_front(ci, st):
        lo, hi = CHUNKS[ci]
        cw = hi - lo
        xs = st["xs"][ci]
        psh = [psL.tile([128, 512], F32, name=f"psh{ci}j{j}{r}", tag="l1")
               for j in (0, 1)]
        for jout in (0, 1):
            for cin in (0, 1):
                nc.tensor.matmul(psh[jout][:, 0:cw], w1blk(cin, jout),
                                 xs[:, cin * cw:(cin + 1) * cw],
                                 start=(cin == 0), stop=(cin == 1))
        h = hpool.tile([128, 1024], BF16, name=f"h{ci}{r}", tag="h")
        nc.scalar.activation(h[:, 0:cw], psh[0][:, 0:cw], AF.Relu,
                             bias=bbt[:, 0:1])
        nc.vector.tensor_scalar(h[:, cw:2 * cw], psh[1][:, 0:cw],
                                bbt[:, 1:2], 0.0, ALU.add, ALU.max)
        st["h"][ci] = h

    def emit_back(ci, st):
        lo, hi = CHUNKS[ci]
        cw = hi - lo
        h = st["h"][ci]
        psz = [psM.tile([128, 512], F32, name=f"psz{ci}j{j}{r}", tag="l2")
               for j in (0, 1)]
        for jout in (0, 1):
            for cin in (0, 1):
                nc.tensor.matmul(psz[jout][:, 0:cw], w2blk(cin, jout),
                                 h[:, cin * cw:(cin + 1) * cw],
                                 start=(cin == 0), stop=(cin == 1))
        # drains: z = psz + b2 (Pool cannot read PSUM; split ACT/DVE)
        if ci < 2:
            nc.scalar.activation(z[:, lo:lo + cw], psz[0][:, 0:cw],
                                 AF.Identity, bias=bbt[:, 2:3])
        else:
            nc.vector.tensor_scalar_add(z[:, lo:lo + cw], psz[0][:, 0:cw],
                                        bbt[:, 2:3])
        nc.vector.tensor_scalar_add(z[:, NC + lo:NC + lo + cw],
                                    psz[1][:, 0:cw], bbt[:, 3:4])

    def emit_norm(ci):
        lo, hi = CHUNKS[ci]
        cw = hi - lo
        # z slices for both jt halves as one strided 3D AP
        zv = z[:].rearrange("p (j c) -> p j c", j=2)[:, :, lo:lo + cw]
        sq = sqpool.tile([128, 1024], BF16, name=f"sq{ci}{r}", tag="sq")
        nc.vector.tensor_mul(
            sq[:, 0:2 * cw].rearrange("p (j c) -> p j c", j=2), zv, zv)
        psn = psM.tile([128, 512], F32, name=f"pn{ci}{r}", tag="pn", bufs=1)
        nc.tensor.matmul(psn[:, 0:cw], ones128[:], sq[:, 0:cw],
                         start=True, stop=False)
        nc.tensor.matmul(psn[:, 0:cw], ones128[:], sq[:, cw:2 * cw],
                         start=False, stop=True)
        lnn = lnpool.tile([128, 512], F32, name=f"ln{ci}{r}", tag="lnn")
        nc.scalar.activation(lnn[:, 0:cw], psn[:, 0:cw], AF.Ln, bias=epsb[:])
        nc.scalar.activation(rn[:, lo:lo + cw], lnn[:, 0:cw], AF.Exp,
                             scale=-0.5)
        rnb = rn[:, lo:lo + cw].unsqueeze(1).to_broadcast([128, 2, cw])
        nc.vector.tensor_mul(
            ph[:].rearrange("p (j c) -> p j c", j=2)[:, :, lo:lo + cw],
            zv, rnb)

    st = {"xs": {}, "h": {}}
    for ci, (lo, hi) in enumerate(CHUNKS):
        cw = hi - lo
        xs = xpool.tile([128, 1024], BF16, name=f"xs{ci}{r}", tag="xs")
        nc.sync.dma_start(xs[:, 0:2 * cw], xt_d[:, 2 * lo:2 * lo + 2 * cw])
        st["xs"][ci] = xs
    if emit_w2:
        # second weight half ordered after the x chunks so chunk-0 compute
        # starts as early as possible; w2 is not needed until L2-c0.
        nc.sync.dma_start(wt[:, 512:1024], wp_d[:, 512:1024])
    if emit_w2:
        # PE p-state warmup: dummy matmuls filling the first-body DMA wait
        # so projection starts at full clock (2.4GHz needs 3us busy ramp).
        warm = psS.tile([100, 256], F32, name=f"warm{r}", tag="sim")
        for i in range(40):
            nc.tensor.matmul(warm[0:100, 0:128], ones128[:, 0:100],
                             ones128[:], start=True, stop=True)
    psp = None
    for t in range(NCH + 2):
        if t < NCH:
            emit_front(t, st)
        if 0 <= t - 1 < NCH:
            emit_back(t - 1, st)
        if 0 <= t - 2 < NCH:
            emit_norm(t - 2)
            if t - 2 == 0:
                # anchors all live in chunk 0: all early sims now
                for si in range(4):
                    emit_sims_mm(si, late=False)
            elif t - 2 == 1:
                emit_pr(0)                # A pos spans chunks 0-1
                emit_pr(1)                # B pos inside chunk 1
                emit_sims_exp(late=False)
            elif t - 2 == 2:
                emit_pr(2)                # C pos spans chunks 1-2
                emit_pr(3)                # D pos inside chunk 2
                # pos col-sums (accumulate jt0+jt1); A then BCD
                ppa = psS.tile([1, 500], F32, name=f"ppa{r}", tag="sim")
                nc.tensor.matmul(ppa[:, 0:500], ones128[:, 0:1],
                                 pr[:, 0:500], start=True, stop=False)
                nc.tensor.matmul(ppa[:, 0:500], ones128[:, 0:1],
                                 pr[:, PRW:PRW + 500], start=False,
                                 stop=True)
                nc.scalar.activation(eprA[:], ppa[:, 0:500], AF.Exp,
                                     scale=2.0)
                ppr = psS.tile([1, 400], F32, name=f"ppr{r}", tag="sim")
                nc.tensor.matmul(ppr[:, 0:400], ones128[:, 0:1],
                                 pr[:, 500:900], start=True, stop=False)
                nc.tensor.matmul(ppr[:, 0:400], ones128[:, 0:1],
                                 pr[:, PRW + 500:PRW + 900], start=False,
                                 stop=True)
                nc.scalar.activation(eprR[:], ppr[:, 0:400], AF.Exp,
                                     scale=2.0)
                # pos sums: 5 rank-1 transposes per slot into psp[100,4]
                # (lives in the l1 tag rotation: slot frees after relu-c3)
                psp = psL.tile([100, 4], F32, name=f"psp{r}", tag="l1")
                for si, sl in enumerate(SLOTS):
                    Sa, eb = sl[2], sl[9]
                    src = eprA if si == 0 else eprR
                    for xy in range(5):
                        nc.tensor.matmul(
                            psp[0:Sa, si:si + 1],
                            src[0:1, eb + xy * Sa:eb + (xy + 1) * Sa],
                            ones11[:], start=(xy == 0), stop=(xy == 4))
                # den partial: early negs + pos (late negs still missing)
                nc.vector.tensor_add(nst[:], nse[:], psp[:])
                # ln(pos) now -- frees the psp psum slot early
                nc.scalar.activation(lnpos[:], psp[:], AF.Ln)

    # ---- after the negs chunk: late sims + final loss ----
    for si in range(4):
        emit_sims_mm(si, late=True)
    emit_sims_exp(late=True)
    nc.vector.tensor_add(den[:], nst[:], nsl[:])
    nc.scalar.activation(lnden[:], den[:], AF.Ln, bias=ne2b[0:100, :])
    nc.vector.tensor_sub(lossv[:], lnden[:], lnpos[:])
    # per-anchor losses go back to the host, which does the final masked
    # reduction (junk rows beyond each slot's Sa are simply not summed).
    nc.sync.dma_start(out_d[:], lossv[:])


def _build_nc(reps=1, loop_iters=None, unroll=1):
    nc = bacc.Bacc("TRN2", target_bir_lowering=False, debug=False)

    xt_d = nc.dram_tensor("xt", [128, 2 * NC], BF16, kind="ExternalInput")
    wp_d = nc.dram_tensor("wp", [128, 1024], BF16, kind="ExternalInput")
    bb_d = nc.dram_tensor("bb", [128, 4], F32, kind="ExternalInput")
    out_d = nc.dram_tensor("out", [100, 4], F32, kind="ExternalOutput")

    with tile.TileContext(nc) as tc:
        with tc.tile_pool(name="const", bufs=1) as cpool:
            bbt = cpool.tile([128, 4], F32, name="bb", tag="bb")
            nc.sync.dma_start(bbt[:], bb_d[:])
            wt = cpool.tile([128, 1024], BF16, name="wt", tag="wt")
            nc.sync.dma_start(wt[:, 0:512], wp_d[:, 0:512])
            ones128 = cpool.tile([128, 128], BF16, name="o128", tag="o128")
            nc.gpsimd.memset(ones128[:], 1.0)
            onesf = cpool.tile([128, 1], F32, name="of", tag="of")
            nc.gpsimd.memset(onesf[:], 1.0)
            ones11 = cpool.tile([1, 1], F32, name="o11", tag="o11")
            nc.gpsimd.memset(ones11[:], 1.0)
            epsb = cpool.tile([128, 1], F32, name="epsb", tag="epsb")
            nc.gpsimd.memset(epsb[:], 1e-24)
            ne2b = cpool.tile([128, 1], F32, name="ne2b", tag="ne2b")
            nc.gpsimd.memset(ne2b[:], -E2)

            consts = (wt, bbt, ones128, onesf, ones11, epsb, ne2b)
            dram = (xt_d, wp_d, out_d)
            with (
                tc.tile_pool(name="xin", bufs=6) as xpool,
                tc.tile_pool(name="hbuf", bufs=2) as hpool,
                tc.tile_pool(name="zbuf", bufs=2) as zpool,
                tc.tile_pool(name="sqbuf", bufs=2) as sqpool,
                tc.tile_pool(name="ssbuf", bufs=2) as sspool,
                tc.tile_pool(name="lnbuf", bufs=2) as lnpool,
                tc.tile_pool(name="rnbuf", bufs=2) as rnpool,
                tc.tile_pool(name="phbuf", bufs=2) as phpool,
                tc.tile_pool(name="prbuf", bufs=2) as prpool,
                tc.tile_pool(name="escb", bufs=2) as escpool,
                tc.tile_pool(name="small", bufs=2) as smallpool,
                tc.tile_pool(name="psL", bufs=3,
                             space=bass.MemorySpace.PSUM) as psL,
                tc.tile_pool(name="psM", bufs=2,
                             space=bass.MemorySpace.PSUM) as psM,
                tc.tile_pool(name="psS", bufs=2,
                             space=bass.MemorySpace.PSUM) as psS,
            ):
                pools = (xpool, hpool, zpool, sqpool, sspool, lnpool, rnpool,
                         phpool, prpool, escpool, smallpool, psL, psM, psS)

                if loop_iters is not None:
                    nc.sync.dma_start(wt[:, 512:1024], wp_d[:, 512:1024])
                    with tc.For_i(0, loop_iters, 1,
                                  hint_engines=(mybir.EngineType.PE,
                                                mybir.EngineType.DVE,
                                                mybir.EngineType.Activation)):
                        for u in range(unroll):
                            _emit_body(nc, tc, consts, dram, pools, u)
                else:
                    for rep in range(reps):
                        _emit_body(nc, tc, consts, dram, pools, rep,
                                   emit_w2=(rep == 0))

    nc.compile()
    return nc


def _get_nc(reps=1, loop_iters=None, unroll=1):
    key = ("nc", reps, loop_iters, unroll)
    if key not in _CACHE:
        _CACHE[key] = _build_nc(reps, loop_iters, unroll)
    return _CACHE[key]


def _assignments():
    """Per-core (A_full_prio, (B_group, half), C_full_rest, (D_group, half))."""
    P = [(t, v, l) for t in (0, 1) for v in range(V) for l in range(L)]
    R = [(t, v, l) for t in (2, 3) for v in range(V) for l in range(L)]
    out = []
    for c in range(NCORES):
        out.append((P[c], (P[8 + c // 2], c % 2), R[c], (R[8 + c // 2], c % 2)))
    return out


def make_in_maps(node_embeddings, W1, b1, W2, b2, idx_prio, idx_rest,
                 neg_idx_prio, neg_idx_rest):
    E = np.asarray(node_embeddings, dtype=np.float32)
    W1 = np.asarray(W1, dtype=np.float32)
    b1 = np.asarray(b1, dtype=np.float32)
    W2 = np.asarray(W2, dtype=np.float32)
    b2 = np.asarray(b2, dtype=np.float32)
    idxp = np.asarray(idx_prio)
    idxr = np.asarray(idx_rest)
    nidxp = np.asarray(neg_idx_prio)
    nidxr = np.asarray(neg_idx_rest)

    # weight blocks: [w1(c0,j0)|w1(c1,j0)|w1(c0,j1)|w1(c1,j1)|w2 same]
    w1t, w2t = W1.T, W2.T
    blocks = []
    for wt_ in (w1t, w2t):
        for jout in (0, 1):
            for cin in (0, 1):
                blocks.append(wt_[cin * 128:(cin + 1) * 128,
                                  jout * 128:(jout + 1) * 128])
    wp = np.concatenate(blocks, axis=1).astype(NP_BF16)
    bbm = np.stack([b1[:128], b1[128:], b2[:128], b2[128:]], axis=1)
    bbm = np.ascontiguousarray(bbm, dtype=np.float32)

    def others(t):
        return [u for u in range(T) if u != t]

    in_maps = []
    for (gA, (gB, hB), gC, (gD, hD)) in _assignments():
        # per-slot (t, v, l, own_ids, oth_ids, negs[3, K])
        slots = []
        t, v, l = gA
        ids = np.asarray(idxp[t][v, l])
        slots.append((t, v, l, ids, None, nidxp[t][v, l]))
        t, v, l = gB
        idf = np.asarray(idxp[t][v, l])
        slots.append((t, v, l, idf[hB * 50:hB * 50 + 50],
                      idf[(1 - hB) * 50:(1 - hB) * 50 + 50], nidxp[t][v, l]))
        t, v, l = gC
        ids = np.asarray(idxr[t - 2][v, l])
        slots.append((t, v, l, ids, None, nidxr[t - 2][v, l]))
        t, v, l = gD
        idf = np.asarray(idxr[t - 2][v, l])
        slots.append((t, v, l, idf[hD * 10:hD * 10 + 10],
                      idf[(1 - hD) * 10:(1 - hD) * 10 + 10],
                      nidxr[t - 2][v, l]))
        # layout: [anchors(+partners) 240 | pos 900 | negs 360]
        anch, pos, negs = [], [], []
        for (t, v, l, own, oth, nvl) in slots:
            anch.append(E[t, v, l, own])
            if oth is not None:
                anch.append(E[t, v, l, np.asarray(oth)])
            for x in range(V):
                for y in range(L):
                    if (x, y) != (v, l):
                        pos.append(E[t, x, y, own])
            for oi, u in enumerate(others(t)):
                negs.append(E[u, v, l, np.asarray(nvl[oi])])
        X = np.concatenate(anch + pos + negs, axis=0)   # [1500, 256]
        assert X.shape == (NC, D), X.shape
        XT = X.T.astype(NP_BF16)                    # [256, 1500]
        xt = np.empty((128, 2 * NC), NP_BF16)
        for lo, hi in CHUNKS:
            cw = hi - lo
            for jt in (0, 1):
                xt[:, 2 * lo + jt * cw:2 * lo + (jt + 1) * cw] = \
                    XT[jt * 128:(jt + 1) * 128, lo:hi]
        in_maps.append({"xt": np.ascontiguousarray(xt), "wp": wp, "bb": bbm})
    return in_maps


def _make_runner(nc):
    """Lower nc to a cached jitted SPMD executable."""
    import jax
    from jax.experimental.shard_map import shard_map
    from jax.sharding import Mesh, PartitionSpec

    from concourse import bass2jax
    from concourse import mybir as mb

    bass2jax.install_neuronx_cc_hook()
    partition_name = (nc.partition_id_tensor.name
                      if nc.partition_id_tensor else None)
    in_names, out_names, out_avals = [], [], []
    for alloc in nc.m.functions[0].allocations:
        if not isinstance(alloc, mb.MemoryLocationSet):
            continue
        name = alloc.memorylocations[0].name
        if alloc.kind == "ExternalInput":
            if name != partition_name:
                in_names.append(name)
        elif alloc.kind == "ExternalOutput":
            out_names.append(name)
            out_avals.append(jax.core.ShapedArray(
                tuple(alloc.tensor_shape), mb.dt.np(alloc.dtype)))
    n_params = len(in_names)
    n_outs = len(out_avals)
    all_in_names = list(in_names) + list(out_names)
    if partition_name is not None:
        all_in_names.append(partition_name)

    def _body(*args):
        operands = list(args)
        if partition_name is not None:
            operands.append(bass2jax.partition_id_tensor())
        return tuple(bass2jax._bass_exec_p.bind(
            *operands,
            out_avals=tuple(out_avals),
            in_names=tuple(all_in_names),
            out_names=tuple(out_names),
            lowering_input_output_aliases=(),
            sim_require_finite=True,
            sim_require_nnan=True,
            nc=nc,
        ))

    devices = jax.devices()[:NCORES]
    mesh = Mesh(np.asarray(devices), ("core",))
    donate = tuple(range(n_params, n_params + n_outs))
    sharded = jax.jit(
        shard_map(_body, mesh=mesh,
                  in_specs=(PartitionSpec("core"),) * (n_params + n_outs),
                  out_specs=(PartitionSpec("core"),) * n_outs,
                  check_rep=False),
        donate_argnums=donate, keep_unused=True)

    def run(in_maps, device_inputs=None):
        if device_inputs is None:
            device_inputs = [
                np.concatenate([np.asarray(m[name]) for m in in_maps], axis=0)
                for name in in_names]
        zeros = [np.zeros((NCORES * a.shape[0], *a.shape[1:]), a.dtype)
                 for a in out_avals]
        out_arrs = sharded(*device_inputs, *zeros)
        return [
            {name: np.asarray(out_arrs[i]).reshape(NCORES, *out_avals[i].shape)[c]
             for i, name in enumerate(out_names)}
            for c in range(NCORES)
        ]

    run.in_names = in_names
    run.mesh = mesh
    return run


def _get_runner(reps=1, loop_iters=None, unroll=1):
    key = ("runner", reps, loop_iters, unroll)
    if key not in _CACHE:
        _CACHE[key] = _make_runner(_get_nc(reps, loop_iters, unroll))
    return _CACHE[key]


class _Res:
    def __init__(self, results):
        self.results = results


def run_on_hw(in_maps, reps=1, device_inputs=None, loop_iters=None,
              unroll=1):
    runner = _get_runner(reps, loop_iters, unroll)
    return _Res(runner(in_maps, device_inputs=device_inputs))


def kernel(node_embeddings, W1, b1, W2, b2, idx_prio, idx_rest,
           neg_idx_prio, neg_idx_rest, num_views=2, num_layers=3):
    in_maps = make_in_maps(node_embeddings, W1, b1, W2, b2, idx_prio,
                           idx_rest, neg_idx_prio, neg_idx_rest)
    res = run_on_hw(in_maps)
    _CACHE["last_results"] = res
    total = 0.0
    for c in range(NCORES):
        lv = np.asarray(res.results[c]["out"], dtype=np.float64)
        for si, sl in enumerate(SLOTS):
            total += lv[0:sl[2], si].sum()
    return np.float32(total / COUNT)
